# revision 31
# baseline (speedup 1.0000x reference)
# Multi-head attention (B=2, S=2048, D=1024, H=16, dh=64) on 8 TRN2 NeuronCores.
# Sharding: core = batch * 4 + head_group; each core handles one batch and 4 heads.
# v4: 2 k-strips of 8 k-tiles, software pipelined. Prologue computes Q(0) and
# K/V for tiles 0-1; strip kk computes scores+exp+PV for q-tiles j >= 2kk,
# accumulating unnormalized attention plus softmax denominators (ones-augmented
# V, M=65) into 65-partition SBUF accumulators with a single tensor_tensor per
# psum half. Remaining Q/K/V projections and Wo chunks interleave as PE filler
# between exp-dependent PV matmuls so TensorE never idles (HAM stays 8/8) and
# ScalarE's exp stream hides underneath. Causal masking is one shared
# triangular [128,2x128] multiply on the boundary block of diagonal tiles.
# Outputs evacuate via ScalarE to bf16 and partial sums are combined on host.
# All host-side tensors are packed partition-major for contiguous descriptors.
import numpy as np
import ml_dtypes

from collections import deque

import concourse.bass as bass
import concourse.tile as tile
from concourse import bacc, mybir
from concourse import bass_utils

B, S, D = 2, 2048, 1024
H, DH = 16, 64
NCORES = 8
GROUPS = 4            # head groups per batch (cores per batch)
HPG = 4               # heads per group
FPG = HPG * DH        # 256 features per group
SQ_T, SK_T = 512, 128
NSQ, NSK = S // SQ_T, S // SK_T
NCH = D // 128        # 8 contraction chunks of d_model
BF16 = ml_dtypes.bfloat16

_BUILT = {}


def _keep_block(mask):
    """Verify causal structure; masking reduces to one shared upper-triangular
    [128,128] block on the boundary column-block of each diagonal tile."""
    keep_t = (~np.asarray(mask, dtype=bool)).T  # [k, q], True = attend
    blk = np.triu(np.ones((SK_T, SK_T), np.float32)).astype(BF16)
    for j in range(NSQ):
        for i in range(NSK):
            sub = keep_t[i * SK_T:(i + 1) * SK_T, j * SQ_T:(j + 1) * SQ_T]
            if i >= 4 * (j + 1):
                assert not sub.any()
            elif i >= 4 * j:  # diagonal tile: boundary block + kept tail
                c0 = (i - 4 * j) * SK_T
                assert (sub[:, c0:c0 + SK_T] == (blk != 0)).all()
                assert sub[:, c0 + SK_T:].all()
                assert not sub[:, :c0].any()
            else:
                assert sub.all()
    # duplicated along a middle dim so both heads mask in one tensor_tensor
    return np.ascontiguousarray(np.stack([blk, blk], axis=1))  # [128, 2, 128]


def _build():
    nc = bacc.Bacc("TRN2", target_bir_lowering=False, debug=False)
    dt = mybir.dt
    f32, bf = dt.float32, dt.bfloat16
    EXP = mybir.ActivationFunctionType.Exp
    MUL = mybir.AluOpType.mult
    ADD = mybir.AluOpType.add

    # all inputs packed partition-major on host: [128, c, s] contiguous
    xq = nc.dram_tensor("xqt", [128, NCH, S], bf, kind="ExternalInput").ap()
    xk = nc.dram_tensor("xkt", [128, NCH, S], bf, kind="ExternalInput").ap()
    xv = nc.dram_tensor("xvt", [128, NCH, S], bf, kind="ExternalInput").ap()
    wq = nc.dram_tensor("wqt", [128, NCH, FPG], bf, kind="ExternalInput").ap()
    wk = nc.dram_tensor("wkt", [128, NCH, FPG], bf, kind="ExternalInput").ap()
    wv = nc.dram_tensor("wvt", [128, NCH, FPG], bf, kind="ExternalInput").ap()
    wo = nc.dram_tensor("wot", [128, FPG // 128, D], bf,
                        kind="ExternalInput").ap()
    kp = nc.dram_tensor("keep", [SK_T, 2, SK_T], bf,
                        kind="ExternalInput").ap()
    out = nc.dram_tensor("out", [S, D], bf, kind="ExternalOutput").ap()
    out_v = out.rearrange("(r p) o -> r p o", p=128)

    with tile.TileContext(nc) as tc:
        with (
            tc.tile_pool(name="consts", bufs=1) as consts,
            tc.tile_pool(name="x", bufs=2) as xpool,
            tc.tile_pool(name="sc", bufs=2, space="PSUM") as sc_ps,
            tc.tile_pool(name="pv", bufs=2, space="PSUM") as pv_ps,
            tc.tile_pool(name="aux", bufs=2, space="PSUM") as aux_ps,
            tc.tile_pool(name="work", bufs=4) as work,
            tc.tile_pool(name="probs", bufs=6) as prpool,
        ):
            # ---------------- SBUF constants ----------------
            wq_sb = consts.tile([128, NCH, FPG], bf)
            wk_sb = consts.tile([128, NCH, FPG], bf)
            wv_sb = consts.tile([128, NCH, FPG], bf)
            wo_sb = consts.tile([128, FPG // 128, D], bf)
            keep_sb = consts.tile([128, 2, SK_T], bf)
            ones_sb = consts.tile([128, 128], bf)
            xq_sb = consts.tile([128, NCH, S], bf)       # full Q input
            qh_sb = consts.tile([128, 2, S], bf)
            kh_sb = consts.tile([128, 2, S], bf)
            vh_sb = consts.tile([128, NSK, HPG, DH + 1], bf)
            # per-pv-half accumulators: rows 0-63 att, row 64 denominator
            attA = consts.tile([DH + 1, 2, S], f32)
            attB = consts.tile([DH + 1, 2, S], f32)
            att_sb = consts.tile([128, 2, S], bf)        # normalized (Wo in)
            # per-hp denominator/reciprocal rows {0,32} (base-0: the custom
            # reciprocal op misbehaves at nonzero base partitions on HW)
            l4 = [consts.tile([64, SQ_T], f32, name=f"l4_{h}")
                  for h in range(2)]
            r4f = [consts.tile([64, SQ_T], f32, name=f"r4f_{h}")
                   for h in range(2)]
            r4 = [consts.tile([64, SQ_T], bf, name=f"r4_{h}")
                  for h in range(2)]

            # ------- initial DMAs: first working set leads each queue -----
            nc.sync.dma_start(wq_sb[:, 0:2, :], wq[:, 0:2, :])
            nc.sync.dma_start(xq_sb[:, 0:2, 0:SQ_T], xq[:, 0:2, 0:SQ_T])
            nc.sync.dma_start(wq_sb[:, 2:8, :], wq[:, 2:8, :])
            nc.sync.dma_start(xq_sb[:, 2:8, 0:SQ_T], xq[:, 2:8, 0:SQ_T])
            nc.scalar.dma_start(wk_sb[:, 0:4, :], wk[:, 0:4, :])
            nc.scalar.dma_start(wk_sb[:, 4:8, :], wk[:, 4:8, :])
            xk_t, xv_t = {}, {}

            def dma_kv(t, queue):
                sl = bass.ts(t, SQ_T)
                xkt = xpool.tile([128, NCH, SQ_T], bf, tag="xk")
                queue.dma_start(xkt[:], xk[:, :, sl])
                xvt = xpool.tile([128, NCH, SQ_T], bf, tag="xv")
                queue.dma_start(xvt[:], xv[:, :, sl])
                xk_t[t], xv_t[t] = xkt, xvt

            dma_kv(0, nc.scalar)
            nc.scalar.dma_start(keep_sb[:], kp)
            nc.scalar.dma_start(wv_sb[:], wv[:])
            for t in range(1, NSQ):
                nc.sync.dma_start(xq_sb[:, :, bass.ts(t, SQ_T)],
                                  xq[:, :, bass.ts(t, SQ_T)])
            nc.sync.dma_start(wo_sb[:], wo[:])

            nc.vector.memset(ones_sb[:], 1.0)
            nc.gpsimd.memset(vh_sb[:], 1.0)  # ones column (col DH) survives
            nc.gpsimd.memset(l4[0][:], 1.0)  # unused partitions -> r = 1
            nc.gpsimd.memset(l4[1][:], 1.0)
            # warm up the ACT exp table under the projection phase
            warm = work.tile([1, 8], f32, tag="warm")
            nc.scalar.activation(warm[:], ones_sb[0:1, 0:8], EXP, scale=0.0)
            # warm up the PE clock (HAM) while the first DMAs land: ~3.5us of
            # dummy matmuls with no DMA dependency, abandoned in PSUM
            wps = aux_ps.tile([128, SQ_T], f32, tag="aux")
            for r in range(26):
                nc.tensor.matmul(wps[:, 0:128], ones_sb[:], ones_sb[:],
                                 start=(r == 0), stop=(r == 25))

            # ---------------- projection closures ----------------
            def qk_closures(t, wsb, xin_f, hout, full_x):
                """4 closures: (hp=0 c0-3, hp=0 c4-7+evac, hp=1 ...)."""
                sl = bass.ts(t, SQ_T)
                cl = []
                for hp in range(2):
                    hsl = bass.ts(hp, 128)
                    box = {}

                    def xsl(xin, c):
                        return xin[:, c, sl] if full_x else xin[:, c, :]

                    def first(hsl=hsl, box=box, wsb=wsb, xin_f=xin_f,
                              xsl=xsl):
                        ps = aux_ps.tile([128, SQ_T], f32, tag="aux")
                        xin = xin_f()
                        for c in range(2):
                            nc.tensor.matmul(ps[:], wsb[:, c, hsl],
                                             xsl(xin, c),
                                             start=(c == 0), stop=False)
                        box["ps"] = ps

                    def second(hp=hp, hsl=hsl, box=box, sl=sl, wsb=wsb,
                               xin_f=xin_f, hout=hout, xsl=xsl):
                        ps = box["ps"]
                        xin = xin_f()
                        for c in range(2, NCH):
                            nc.tensor.matmul(ps[:], wsb[:, c, hsl],
                                             xsl(xin, c),
                                             start=False, stop=(c == NCH - 1))
                        nc.vector.tensor_copy(hout[:, hp, sl], ps[:])

                    cl += [first, second]
                return cl

            def v_closures(t):
                """8 closures: per s4 subtile (c0-3, c4-7+evac)."""
                cl = []
                for s4 in range(SQ_T // SK_T):
                    i = t * (SQ_T // SK_T) + s4
                    box = {}

                    def vfirst(s4=s4, box=box, t=t):
                        ps = aux_ps.tile([128, SQ_T], f32, tag="aux")
                        for c in range(4):
                            nc.tensor.matmul(ps[:, 0:FPG],
                                             xv_t[t][:, c, bass.ts(s4, SK_T)],
                                             wv_sb[:, c, :],
                                             start=(c == 0), stop=False)
                        box["ps"] = ps

                    def vsecond(s4=s4, box=box, t=t, i=i):
                        ps = box["ps"]
                        for c in range(4, NCH):
                            nc.tensor.matmul(ps[:, 0:FPG],
                                             xv_t[t][:, c, bass.ts(s4, SK_T)],
                                             wv_sb[:, c, :],
                                             start=False, stop=(c == NCH - 1))
                        nc.vector.tensor_copy(
                            vh_sb[:, i, :, 0:DH],
                            ps[:, 0:FPG].rearrange("p (h d) -> p h d", h=HPG))

                    cl += [vfirst, vsecond]
                return cl

            def wo_closures(j):
                """8 closures: per (t4, o) a 2-MM accumulation + evac + DMA."""
                cl = []
                for t4 in range(SQ_T // 128):
                    r_ = j * (SQ_T // 128) + t4
                    tsl = bass.ds(j * SQ_T + t4 * 128, 128)
                    for o in range(2):
                        def wone(r_=r_, tsl=tsl, o=o):
                            po = aux_ps.tile([128, SQ_T], f32, tag="aux")
                            for hp in range(2):
                                nc.tensor.matmul(po[:], att_sb[:, hp, tsl],
                                                 wo_sb[:, hp, bass.ts(o, 512)],
                                                 start=(hp == 0),
                                                 stop=(hp == 1))
                            ost = work.tile([128, 512], bf, tag="ost")
                            nc.vector.tensor_copy(ost[:], po[:])
                            # last q-tile: inputs done, use both HWDGE rings
                            q = nc.scalar if (j == NSQ - 1 and o == 1) \
                                else nc.sync
                            q.dma_start(out_v[r_, :, bass.ts(o, 512)],
                                        ost[:])
                        cl.append(wone)
                return cl

            # ---------------- filler machinery ----------------
            fillers = deque()  # (tag, closure)

            def drain_tag(tag):
                keep = deque()
                while fillers:
                    tg, fn = fillers.popleft()
                    if tg == tag:
                        fn()
                    else:
                        keep.append((tg, fn))
                fillers.extend(keep)

            def pop_fillers(n):
                for _ in range(n):
                    if not fillers:
                        return
                    _, fn = fillers.popleft()
                    fn()

            # ------- prologue: Q(0), K/V strip 0; rest becomes filler -----
            for fn in qk_closures(0, wq_sb, lambda: xq_sb, qh_sb, True):
                fn()
            dma_kv(1, nc.scalar)
            for fn in qk_closures(0, wk_sb, lambda: xk_t[0], kh_sb, False):
                fn()
            for fn in v_closures(0):
                fn()
            for t in range(1, NSQ):
                for fn in qk_closures(t, wq_sb, lambda: xq_sb, qh_sb, True):
                    fillers.append((("q", t), fn))

            # ---------------- k-strips (4 strips of 4 k-tiles) ------------
            for t in range(NSQ):
                if t + 1 < NSQ:
                    if t + 2 < NSQ:
                        dma_kv(t + 2, nc.scalar)
                    for fn in qk_closures(t + 1, wk_sb,
                                          (lambda t_: lambda: xk_t[t_])(t + 1),
                                          kh_sb, False):
                        fillers.append((("kv", t + 1), fn))
                    for fn in v_closures(t + 1):
                        fillers.append((("kv", t + 1), fn))
                drain_tag(("kv", t))  # ensure this strip's K/V emitted
                positions = (NSQ - t) * 2 * 4
                for j in range(t, NSQ):
                    drain_tag(("q", j))
                    jsl = bass.ts(j, SQ_T)
                    diagstrip = (j == t)
                    for hp in range(2):
                        pv0 = pv_ps.tile([DH + 1, SQ_T], f32, tag="pv")
                        pv1 = pv_ps.tile([DH + 1, SQ_T], f32, tag="pv")
                        for n in range(4):
                            i = 4 * t + n
                            isl = bass.ts(i, SK_T)
                            c0 = n * SK_T if diagstrip else 0
                            qsl = bass.ds(j * SQ_T + c0, SQ_T - c0)
                            sc = sc_ps.tile([128, 2, SQ_T], f32, tag="sc")
                            nc.tensor.matmul(sc[:, 0, c0:SQ_T],
                                             kh_sb[0:64, hp, isl],
                                             qh_sb[0:64, hp, qsl], start=True,
                                             stop=True, tile_position=(0, 0))
                            nc.tensor.matmul(sc[:, 1, c0:SQ_T],
                                             kh_sb[64:128, hp, isl],
                                             qh_sb[64:128, hp, qsl],
                                             start=True, stop=True,
                                             tile_position=(64, 0))
                            pr = prpool.tile([128, 2, SQ_T], bf, tag="probs")
                            nc.scalar.activation(pr[:, :, c0:SQ_T],
                                                 sc[:, :, c0:SQ_T],
                                                 EXP, scale=0.125)
                            if diagstrip:
                                # mask the triangular boundary block (1 op)
                                nc.vector.tensor_mul(
                                    pr[:, :, c0:c0 + SK_T],
                                    pr[:, :, c0:c0 + SK_T], keep_sb[:])
                            # paced PE filler between exp and its PV consumer
                            nfill = -(-len(fillers) // max(positions, 1))
                            pop_fillers(min(nfill, 2))
                            positions -= 1
                            nc.tensor.matmul(pv0[:, c0:SQ_T],
                                             vh_sb[:, i, 2 * hp + 0, :],
                                             pr[:, 0, c0:SQ_T],
                                             start=(n == 0), stop=(n == 3))
                            nc.tensor.matmul(pv1[:, c0:SQ_T],
                                             vh_sb[:, i, 2 * hp + 1, :],
                                             pr[:, 1, c0:SQ_T],
                                             start=(n == 0), stop=(n == 3))
                        if diagstrip:
                            # denominator chain first so the rb matmul
                            # unblocks before the big att evacuations; for
                            # the final strip read l straight from PSUM.
                            # Rows {0,32}, base 0 (recip_approx breaks
                            # off-base-0 on HW).
                            if t == 0:
                                nc.vector.tensor_copy(l4[hp][0:1, :],
                                                      pv0[DH:DH + 1, :])
                                nc.vector.tensor_copy(l4[hp][32:33, :],
                                                      pv1[DH:DH + 1, :])
                            else:
                                nc.vector.tensor_tensor(
                                    l4[hp][0:1, :], pv0[DH:DH + 1, :],
                                    attA[DH:DH + 1, hp, jsl], ADD)
                                nc.vector.tensor_tensor(
                                    l4[hp][32:33, :], pv1[DH:DH + 1, :],
                                    attB[DH:DH + 1, hp, jsl], ADD)
                            nc.vector.reciprocal_approx_fast(r4f[hp][:],
                                                             l4[hp][:])
                            nc.vector.tensor_copy(r4[hp][:], r4f[hp][:])
                        # single-op evacuation: att rows + denominator row
                        if t == 0:
                            nc.vector.tensor_copy(attA[:, hp, jsl], pv0[:])
                            nc.vector.tensor_copy(attB[:, hp, jsl], pv1[:])
                        else:
                            nc.vector.tensor_tensor(attA[:, hp, jsl], pv0[:],
                                                    attA[:, hp, jsl], ADD)
                            nc.vector.tensor_tensor(attB[:, hp, jsl], pv1[:],
                                                    attB[:, hp, jsl], ADD)
                    if diagstrip:
                        # PE-side broadcast + normalization after both hps
                        for hp in range(2):
                            rb = aux_ps.tile([128, SQ_T], f32, tag="aux")
                            nc.tensor.matmul(rb[0:64, :],
                                             ones_sb[0:1, 0:64],
                                             r4[hp][0:1, :], start=True,
                                             stop=True, tile_position=(0, 0))
                            nc.tensor.matmul(rb[64:128, :],
                                             ones_sb[32:33, 64:128],
                                             r4[hp][32:33, :], start=True,
                                             stop=True,
                                             tile_position=(32, 64))
                            nc.vector.tensor_tensor(att_sb[0:64, hp, jsl],
                                                    attA[0:64, hp, jsl],
                                                    rb[0:64, :], MUL)
                            nc.vector.tensor_tensor(att_sb[64:128, hp, jsl],
                                                    attB[0:64, hp, jsl],
                                                    rb[64:128, :], MUL)
                        for fn in wo_closures(j):
                            fillers.append((("wo", j), fn))
            while fillers:
                _, fn = fillers.popleft()
                fn()

    nc.compile()
    return nc


def _get_nc(mask):
    key = hash(np.asarray(mask, dtype=bool).tobytes())
    if key not in _BUILT:
        blk = _keep_block(mask)
        _BUILT[key] = (_build(), blk)
    return _BUILT[key]


def _pack_cs(x):
    """[rows=c*128, cols] -> partition-major [128, c, cols] contiguous."""
    c = x.shape[0] // 128
    return np.ascontiguousarray(
        x.reshape(c, 128, x.shape[1]).transpose(1, 0, 2))


def _kernel_impl(q, k, v, attn_mask, Wq, Wk, Wv, Wo, trace=False):
    q = np.asarray(q, dtype=np.float32)
    k = np.asarray(k, dtype=np.float32)
    v = np.asarray(v, dtype=np.float32)
    nc, keep_blk = _get_nc(attn_mask)

    xt = {}
    for b in range(B):
        xt[("q", b)] = _pack_cs(q[b].T.astype(BF16))
        xt[("k", b)] = _pack_cs(k[b].T.astype(BF16))
        xt[("v", b)] = _pack_cs(v[b].T.astype(BF16))
    wslices = {}
    for g in range(GROUPS):
        fsl = slice(g * FPG, (g + 1) * FPG)
        wslices[("wq", g)] = _pack_cs(Wq[fsl, :].T.astype(BF16))
        wslices[("wk", g)] = _pack_cs(Wk[fsl, :].T.astype(BF16))
        wslices[("wv", g)] = _pack_cs(Wv[fsl, :].T.astype(BF16))
        wslices[("wo", g)] = _pack_cs(Wo[:, fsl].T.astype(BF16))

    in_maps = []
    for core in range(NCORES):
        b, g = core // GROUPS, core % GROUPS
        in_maps.append({
            "xqt": xt[("q", b)], "xkt": xt[("k", b)], "xvt": xt[("v", b)],
            "wqt": wslices[("wq", g)], "wkt": wslices[("wk", g)],
            "wvt": wslices[("wv", g)], "wot": wslices[("wo", g)],
            "keep": keep_blk,
        })

    res = bass_utils.run_bass_kernel_spmd(
        nc, in_maps, core_ids=list(range(NCORES)), trace=trace)

    out = np.zeros((B, S, D), dtype=np.float32)
    for core in range(NCORES):
        out[core // GROUPS] += res.results[core]["out"].astype(np.float32)
    return out, res


def kernel(q, k, v, attn_mask, Wq, Wk, Wv, Wo):
    out, _ = _kernel_impl(q, k, v, attn_mask, Wq, Wk, Wv, Wo)
    return out
